# revision 18
# baseline (speedup 1.0000x reference)
"""Causal self-attention (B=2,T=2048,D=1024,H=16,HD=64) + RoPE on 8 TRN2 NeuronCores.

Sharding: core = b*4 + g  (b: batch, g: head-group of 4 heads).
Each core computes QKV projection for its 4 heads, causal attention, and a
partial out-projection (rank-256 contribution). Host sums the 4 partials per
batch (the "all-reduce after out_proj").

All matmul operands are bf16 (PSUM accumulation in fp32): bf16 streams at
1 col/cycle on the PE vs 2 for fp32, and enables fast weight loads.

The attention inner loop is software-pipelined at block granularity: PV(j-1)
is issued one iteration late so the PE never waits on the ACT exp, and QKV /
out-projection matmul "fill units" are injected between score and PV issues
to keep the PE stream dense (no HAM re-throttle) while ACT drains exps.
"""
import numpy as np
import ml_dtypes

import concourse.bass as bass
import concourse.mybir as mybir
from concourse import bacc
from concourse.tile import TileContext
from concourse.bass_utils import run_bass_kernel_spmd

B, T, D, H = 2, 2048, 1024, 16
HD = D // H            # 64
G = 4                  # head groups (tensor-parallel factor)
HPG = H // G           # 4 heads per group
DG = HPG * HD          # 256 head-dims per group
KC = D // 128          # 8 contraction chunks for D
NT = T // 512          # 4 T-chunks of 512
TT = T // 128          # 16 T-tiles of 128
F32 = mybir.dt.float32
BF16 = mybir.dt.bfloat16
SWAP16 = [(i + 16) % 32 for i in range(32)]  # e<->o halves within each 32-quadrant

_CACHE = {}


def _build():
    nc = bacc.Bacc("TRN2", target_bir_lowering=False, debug=False, num_devices=8)

    xT_d = nc.dram_tensor("xT", [128, NT, KC, 512], BF16, kind="ExternalInput").ap()
    wqk_d = nc.dram_tensor("wqk", [128, KC, 2 * DG], BF16, kind="ExternalInput").ap()
    wv_d = nc.dram_tensor("wv", [128, KC, DG], BF16, kind="ExternalInput").ap()
    wout_d = nc.dram_tensor("wout", [128, 2, D], BF16, kind="ExternalInput").ap()
    cos_d = nc.dram_tensor("cos128", [128, T], F32, kind="ExternalInput").ap()
    sin_d = nc.dram_tensor("sin128s", [128, T], F32, kind="ExternalInput").ap()
    tri_d = nc.dram_tensor("tri", [128, 128], BF16, kind="ExternalInput").ap()
    out_d = nc.dram_tensor("out", [T, D], F32, kind="ExternalOutput").ap()

    with TileContext(nc) as tc:
        with (
            tc.tile_pool(name="const", bufs=1) as cpool,
            tc.tile_pool(name="big", bufs=1) as big,
            tc.tile_pool(name="work", bufs=2) as work,
            tc.tile_pool(name="expp", bufs=3) as expp,
            tc.tile_pool(name="outp", bufs=2) as outp,
            tc.tile_pool(name="ps_mm", bufs=2, space="PSUM") as ps_mm,
            tc.tile_pool(name="ps_sc", bufs=2, space="PSUM") as ps_sc,
            tc.tile_pool(name="ps_pv", bufs=1, space="PSUM") as ps_pv,
        ):
            cos_sb = cpool.tile([128, T], F32)
            sin_sb = cpool.tile([128, T], F32)
            tri_sb = cpool.tile([128, 128], BF16)
            xT_sb = big.tile([128, KC, T], BF16)
            wqk_sb = big.tile([128, KC, 2 * DG], BF16)
            wv_sb = big.tile([128, KC, DG], BF16)
            wout_sb = big.tile([128, 2, D], BF16)
            # inputs arrive pre-arranged in SBUF layout; first q/k matmuls
            # need wqk + xT chunk 0 (contiguous in DRAM), then RoPE needs
            # cos/sin — everything else after
            nc.sync.dma_start(out=wqk_sb[:], in_=wqk_d)
            nc.sync.dma_start(out=xT_sb[:, :, 0:512], in_=xT_d[:, 0])
            nc.sync.dma_start(out=wv_sb[:], in_=wv_d)
            nc.sync.dma_start(out=cos_sb[:], in_=cos_d[:])
            nc.sync.dma_start(out=sin_sb[:], in_=sin_d[:])
            nc.sync.dma_start(out=tri_sb[:], in_=tri_d)
            for n in range(1, NT):
                nc.sync.dma_start(
                    out=xT_sb[:, :, n * 512:(n + 1) * 512],
                    in_=xT_d[:, n],
                )
            nc.sync.dma_start(out=wout_sb[:], in_=wout_d)

            # PE warm-up: dummy matmuls fill the DMA lead-in so HAM unthrottles
            # before the first real matmul
            warm_sb = cpool.tile([128, 256], BF16)
            nc.vector.memset(warm_sb[:], 0.0)
            for w in range(46):
                wp = ps_sc.tile([128, 256], F32, tag="sc")
                nc.tensor.matmul(
                    wp[:], lhsT=warm_sb[:, 0:128], rhs=warm_sb[:],
                    start=True, stop=True,
                )

            # qkT_sb m-index: 0,1 = q head-pairs (0,1),(2,3); 2,3 = k pairs
            qkT_sb = big.tile([128, 4, T], BF16)
            v_sb = big.tile([128, TT, HPG, HD + 1], BF16)
            nc.vector.memset(v_sb[:, :, :, HD], 1.0)
            outT_sb = big.tile([128, 2, T], BF16)

            def do_qk_tile(n, m):
                ns = slice(n * 512, (n + 1) * 512)
                ps = ps_mm.tile([128, 512], F32, tag="mm")
                for k in range(KC):
                    nc.tensor.matmul(
                        ps[:],
                        lhsT=wqk_sb[:, k, m * 128:(m + 1) * 128],
                        rhs=xT_sb[:, k, ns],
                        start=(k == 0),
                        stop=(k == KC - 1),
                    )
                # RoPE: rot = ps*cos + swap16(ps)*sin_signed — all DVE, fp32;
                # gpsimd stays single-purpose (partition broadcast) so it
                # never swaps ucode libraries
                qk_raw = work.tile([128, 512], F32, tag="qk_raw", bufs=3)
                swp = work.tile([128, 512], F32, tag="swp")
                nc.vector.tensor_copy(qk_raw[:], ps[:])
                nc.vector.stream_shuffle(swp[:], qk_raw[:], SWAP16)
                nc.vector.tensor_mul(qk_raw[:], qk_raw[:], cos_sb[:, ns])
                nc.vector.tensor_mul(swp[:], swp[:], sin_sb[:, ns])
                nc.vector.tensor_add(qkT_sb[:, m, ns], qk_raw[:], swp[:])

            def do_v_tile(n, j):
                ps = ps_mm.tile([128, 256], F32, tag="mm")
                for k in range(KC):
                    nc.tensor.matmul(
                        ps[:],
                        lhsT=xT_sb[:, k, j * 128:(j + 1) * 128],
                        rhs=wv_sb[:, k, :],
                        start=(k == 0),
                        stop=(k == KC - 1),
                    )
                nc.vector.tensor_copy(
                    v_sb[:, j, :, 0:HD], ps[:].rearrange("p (h d) -> p h d", h=HPG)
                )

            def do_proj_tile(t, nh):
                ps = ps_mm.tile([128, 512], F32, tag="mm")
                for c in range(2):
                    nc.tensor.matmul(
                        ps[:],
                        lhsT=outT_sb[:, c, t * 128:(t + 1) * 128],
                        rhs=wout_sb[:, c, nh * 512:(nh + 1) * 512],
                        start=(c == 0),
                        stop=(c == 1),
                    )
                ot = outp.tile([128, 512], F32, tag="ot")
                if (t + nh) % 2 == 0:
                    nc.scalar.copy(out=ot[:], in_=ps[:])
                else:
                    nc.vector.tensor_copy(ot[:], ps[:])
                nc.sync.dma_start(
                    out=out_d[t * 128:(t + 1) * 128, nh * 512:(nh + 1) * 512],
                    in_=ot[:],
                )

            # ---- qkv chunk 0 head-pair 0 up front (attn group 0 hp=0 needs
            # m=0,2 + v tiles; m=1,3 for hp=1 go in as early fill) ----
            do_qk_tile(0, 0)
            do_qk_tile(0, 2)
            for j in range(4):
                do_v_tile(0, j)

            # fill units injected into attention groups, balanced so the
            # ACT-heavy late groups carry less fill
            fills = {
                0: [lambda: do_qk_tile(0, 1), lambda: do_qk_tile(0, 3)]
                 + [lambda m=m: do_qk_tile(1, m) for m in range(4)]
                 + [lambda j=j: do_v_tile(1, j) for j in range(4, 8)],
                1: [lambda m=m: do_qk_tile(2, m) for m in range(4)]
                 + [lambda j=j: do_v_tile(2, j) for j in range(8, 12)],
                2: [lambda m=m: do_qk_tile(3, m) for m in range(4)]
                 + [lambda j=j: do_v_tile(3, j) for j in range(12, 16)]
                 + [lambda t=t, nh=nh: do_proj_tile(t, nh)
                    for t in range(0, 4) for nh in range(2)],
                3: [lambda t=t, nh=nh: do_proj_tile(t, nh)
                    for t in range(4, 12) for nh in range(2)],
            }

            def make_pv(g, hp, j, jmax, ex, pv0, pv1, ncols, nstart):
                def issue():
                    for half, pv in ((0, pv0), (1, pv1)):
                        nc.tensor.matmul(
                            pv[:, nstart:512],
                            lhsT=v_sb[:, j, 2 * hp + half, :],
                            rhs=ex[:, half * 512:half * 512 + ncols],
                            start=(j == 0),
                            stop=(j == jmax),
                        )
                return issue

            def make_fin(g, hp, pv0, pv1):
                def issue():
                    for half, pv in ((0, pv0), (1, pv1)):
                        pb = 64 * half
                        den = work.tile([1, 512], F32, tag="den", bufs=1)
                        nc.vector.tensor_copy(den[:], pv[64:65, :])
                        rec = work.tile([1, 512], F32, tag="rec", bufs=1)
                        nc.vector.reciprocal_approx_fast(rec[:], den[:])
                        recb = work.tile([64, 512], F32, tag="recb", bufs=1)
                        nc.gpsimd.partition_broadcast(recb[:], rec[0:1, :], channels=64)
                        nc.vector.tensor_mul(
                            outT_sb[pb:pb + 64, hp, g * 512:(g + 1) * 512],
                            pv[0:64, :],
                            recb[:],
                        )
                return issue

            pend = []  # closures from the previous slot (PV pair, maybe fin)
            for g in range(4):
                fl = fills[g]
                slots = [(hp, j) for hp in range(2) for j in range(4 * g + 4)]
                fi = 0
                pv_state = {}
                for idx, (hp, j) in enumerate(slots):
                    qm, km = hp, 2 + hp
                    jmax = 4 * g + 3
                    if j == 0:
                        pv0_t = ps_pv.tile([65, 512], F32, tag="pv0", name="pv0")
                        pv1_t = ps_pv.tile([65, 512], F32, tag="pv1", name="pv1")
                        pv_state[hp] = (pv0_t, pv1_t)
                    pv0, pv1 = pv_state[hp]
                    d = j - 4 * g
                    nstart = 128 * d if d > 0 else 0
                    ncols = 512 - nstart
                    ex = expp.tile([128, 1024], BF16, tag="ex")
                    # two heads' score matmuls packed into one PE pass
                    # (row groups 0-1 / 2-3), one wide exp over both
                    sc = ps_sc.tile([128, 1024], F32, tag="sc")
                    for half in range(2):
                        pb = 64 * half
                        nc.tensor.matmul(
                            sc[:, half * 512:half * 512 + ncols],
                            lhsT=qkT_sb[pb:pb + 64, km, j * 128:(j + 1) * 128],
                            rhs=qkT_sb[pb:pb + 64, qm, g * 512 + nstart:(g + 1) * 512],
                            start=True,
                            stop=True,
                        )
                    if ncols == 512:
                        nc.scalar.activation(
                            ex[:], sc[:],
                            mybir.ActivationFunctionType.Exp, scale=0.125,
                        )
                    else:
                        exv = ex[:].rearrange("p (u c) -> p u c", u=2)[:, :, 0:ncols]
                        scv = sc[:].rearrange("p (u c) -> p u c", u=2)[:, :, 0:ncols]
                        nc.scalar.activation(
                            exv, scv, mybir.ActivationFunctionType.Exp, scale=0.125,
                        )
                    if d >= 0:
                        nc.vector.tensor_mul(ex[:, 0:128], ex[:, 0:128], tri_sb[:])
                        nc.vector.tensor_mul(ex[:, 512:640], ex[:, 512:640], tri_sb[:])
                    # fill the PE stream while ACT computes this block's exp
                    while fi < len(fl) and fi <= idx * len(fl) // len(slots):
                        fl[fi]()
                        fi += 1
                    # previous slot's PV (its exp has had a full slot to finish)
                    for c in pend:
                        c()
                    pend = [make_pv(g, hp, j, jmax, ex, pv0, pv1, ncols, nstart)]
                    if j == jmax:
                        pend.append(make_fin(g, hp, pv0, pv1))
                while fi < len(fl):
                    fl[fi]()
                    fi += 1
            for c in pend:
                c()
            for t in range(12, 16):
                for nh in range(2):
                    do_proj_tile(t, nh)

    nc.compile()
    return nc


def _qk_perm():
    """hd permutation for q/k columns: RoPE pair j -> (e,o) rows 16-interleaved
    so the swap stays within 32-partition quadrants (stream_shuffle-able)."""
    perm = np.empty(HD, dtype=np.int64)
    for p in range(HD):
        q32, i = divmod(p, 32)
        j = 16 * q32 + (i % 16)
        perm[p] = 2 * j + (1 if i >= 16 else 0)
    return perm


def _prepare_shards(x, w_qkv, w_out, freqs_cos, freqs_sin):
    perm = _qk_perm()
    cosT = np.ascontiguousarray(freqs_cos.T)  # [32, T]
    sinT = np.ascontiguousarray(freqs_sin.T)
    # row p of a 64-row head block: pair j = 16*(p//32 % 2) + p%16, sign -/+ for e/o
    cos128 = np.empty((128, T), dtype=np.float32)
    sin128s = np.empty((128, T), dtype=np.float32)
    for p in range(128):
        ph = p % 64
        q32, i = divmod(ph, 32)
        j = 16 * q32 + (i % 16)
        cos128[p] = cosT[j]
        sin128s[p] = sinT[j] * (-1.0 if i < 16 else 1.0)
    kk, qq = np.meshgrid(np.arange(128), np.arange(128), indexing="ij")
    tri = (kk <= qq).astype(ml_dtypes.bfloat16)

    w3 = w_qkv.reshape(D, 3, H, HD)
    in_maps = []
    for core in range(8):
        b, g = divmod(core, G)
        heads = np.arange(g * HPG, (g + 1) * HPG)
        wq = w3[:, 0, heads][:, :, perm].reshape(D, DG)
        wk = w3[:, 1, heads][:, :, perm].reshape(D, DG)
        wqk = np.ascontiguousarray(np.concatenate([wq, wk], axis=1))
        wv = np.ascontiguousarray(w3[:, 2, heads].reshape(D, DG))
        wo = np.ascontiguousarray(w_out.reshape(H, HD, D)[heads].reshape(DG, D))
        def sb_layout(a, kc=KC):
            # [128*kc, F] -> [128, kc, F] with partition-major contiguity
            return np.ascontiguousarray(
                a.reshape(kc, 128, -1).transpose(1, 0, 2)
            ).astype(ml_dtypes.bfloat16)
        def xT_layout(a):
            # [D, T] -> [128, NT, KC, 512]: T-chunks contiguous per partition
            t = a.reshape(KC, 128, NT, 512)
            return np.ascontiguousarray(
                t.transpose(1, 2, 0, 3)
            ).astype(ml_dtypes.bfloat16)
        in_maps.append({
            "xT": xT_layout(x[b].T),
            "wqk": sb_layout(wqk),
            "wv": sb_layout(wv),
            "wout": sb_layout(wo, kc=2),
            "cos128": cos128,
            "sin128s": sin128s,
            "tri": tri,
        })
    return in_maps


def _run(in_maps, **kw):
    if "nc" not in _CACHE:
        _CACHE["nc"] = _build()
    return run_bass_kernel_spmd(_CACHE["nc"], in_maps, core_ids=list(range(8)), **kw)


def kernel(x, w_qkv, w_out, freqs_cos, freqs_sin):
    x = np.asarray(x, dtype=np.float32)
    w_qkv = np.asarray(w_qkv, dtype=np.float32)
    w_out = np.asarray(w_out, dtype=np.float32)
    freqs_cos = np.asarray(freqs_cos, dtype=np.float32)
    freqs_sin = np.asarray(freqs_sin, dtype=np.float32)

    in_maps = _prepare_shards(x, w_qkv, w_out, freqs_cos, freqs_sin)
    res = _run(in_maps)
    out = np.zeros((B, T, D), dtype=np.float64)
    for core in range(8):
        out[core // G] += res.results[core]["out"].astype(np.float64)
    return out.astype(np.float32)


# revision 21
# speedup vs baseline: 1.0041x; 1.0041x over previous
"""Causal self-attention (B=2,T=2048,D=1024,H=16,HD=64) + RoPE on 8 TRN2 NeuronCores.

Sharding: core = b*4 + g  (b: batch, g: head-group of 4 heads).
Each core computes QKV projection for its 4 heads, causal attention, and a
partial out-projection (rank-256 contribution). Host sums the 4 partials per
batch (the "all-reduce after out_proj").

All matmul operands are bf16 (PSUM accumulation in fp32): bf16 streams at
1 col/cycle on the PE vs 2 for fp32, and enables fast weight loads.

The attention inner loop is software-pipelined at block granularity: PV(j-1)
is issued one iteration late so the PE never waits on the ACT exp, and QKV /
out-projection matmul "fill units" are injected between score and PV issues
to keep the PE stream dense (no HAM re-throttle) while ACT drains exps.
"""
import numpy as np
import ml_dtypes

import concourse.bass as bass
import concourse.mybir as mybir
from concourse import bacc
from concourse.tile import TileContext
from concourse.bass_utils import run_bass_kernel_spmd

B, T, D, H = 2, 2048, 1024, 16
HD = D // H            # 64
G = 4                  # head groups (tensor-parallel factor)
HPG = H // G           # 4 heads per group
DG = HPG * HD          # 256 head-dims per group
KC = D // 128          # 8 contraction chunks for D
NT = T // 512          # 4 T-chunks of 512
TT = T // 128          # 16 T-tiles of 128
F32 = mybir.dt.float32
BF16 = mybir.dt.bfloat16
SWAP16 = [(i + 16) % 32 for i in range(32)]  # e<->o halves within each 32-quadrant

_CACHE = {}


def _build():
    nc = bacc.Bacc("TRN2", target_bir_lowering=False, debug=False, num_devices=8)

    xT_d = nc.dram_tensor("xT", [128, NT, KC, 512], BF16, kind="ExternalInput").ap()
    wqk_d = nc.dram_tensor("wqk", [128, KC, 2 * DG], BF16, kind="ExternalInput").ap()
    wv_d = nc.dram_tensor("wv", [128, KC, DG], BF16, kind="ExternalInput").ap()
    wout_d = nc.dram_tensor("wout", [128, 2, D], BF16, kind="ExternalInput").ap()
    cos_d = nc.dram_tensor("cos128", [128, T], F32, kind="ExternalInput").ap()
    sin_d = nc.dram_tensor("sin128s", [128, T], F32, kind="ExternalInput").ap()
    tri_d = nc.dram_tensor("tri", [128, 128], BF16, kind="ExternalInput").ap()
    out_d = nc.dram_tensor("out", [T, D], F32, kind="ExternalOutput").ap()

    with TileContext(nc) as tc:
        with (
            tc.tile_pool(name="const", bufs=1) as cpool,
            tc.tile_pool(name="big", bufs=1) as big,
            tc.tile_pool(name="work", bufs=2) as work,
            tc.tile_pool(name="expp", bufs=3) as expp,
            tc.tile_pool(name="outp", bufs=2) as outp,
            tc.tile_pool(name="ps_mm", bufs=2, space="PSUM") as ps_mm,
            tc.tile_pool(name="ps_sc", bufs=2, space="PSUM") as ps_sc,
            tc.tile_pool(name="ps_pv", bufs=1, space="PSUM") as ps_pv,
        ):
            cos_sb = cpool.tile([128, T], F32)
            sin_sb = cpool.tile([128, T], F32)
            tri_sb = cpool.tile([128, 128], BF16)
            xT_sb = big.tile([128, KC, T], BF16)
            wqk_sb = big.tile([128, KC, 2 * DG], BF16)
            wv_sb = big.tile([128, KC, DG], BF16)
            wout_sb = big.tile([128, 2, D], BF16)
            # inputs arrive pre-arranged in SBUF layout; first q/k matmuls
            # need wqk + xT chunk 0 (contiguous in DRAM), then RoPE needs
            # cos/sin — everything else after
            nc.sync.dma_start(out=wqk_sb[:], in_=wqk_d)
            nc.sync.dma_start(out=xT_sb[:, :, 0:512], in_=xT_d[:, 0])
            nc.sync.dma_start(out=wv_sb[:], in_=wv_d)
            nc.sync.dma_start(out=cos_sb[:], in_=cos_d[:])
            nc.sync.dma_start(out=sin_sb[:], in_=sin_d[:])
            nc.sync.dma_start(out=tri_sb[:], in_=tri_d)
            for n in range(1, NT):
                nc.sync.dma_start(
                    out=xT_sb[:, :, n * 512:(n + 1) * 512],
                    in_=xT_d[:, n],
                )
            nc.sync.dma_start(out=wout_sb[:], in_=wout_d)

            # PE warm-up: dummy matmuls fill the DMA lead-in so HAM unthrottles
            # before the first real matmul
            warm_sb = cpool.tile([128, 256], BF16)
            nc.vector.memset(warm_sb[:], 0.0)
            for w in range(46):
                wp = ps_sc.tile([128, 256], F32, tag="sc")
                nc.tensor.matmul(
                    wp[:], lhsT=warm_sb[:, 0:128], rhs=warm_sb[:],
                    start=True, stop=True,
                )

            # qkT_sb m-index: 0,1 = q head-pairs (0,1),(2,3); 2,3 = k pairs
            qkT_sb = big.tile([128, 4, T], BF16)
            v_sb = big.tile([128, TT, HPG, HD + 1], BF16)
            nc.vector.memset(v_sb[:, :, :, HD], 1.0)
            outT_sb = big.tile([128, 2, T], BF16)

            def do_qk_tile(n, m):
                ns = slice(n * 512, (n + 1) * 512)
                ps = ps_mm.tile([128, 512], F32, tag="mm")
                for k in range(KC):
                    nc.tensor.matmul(
                        ps[:],
                        lhsT=wqk_sb[:, k, m * 128:(m + 1) * 128],
                        rhs=xT_sb[:, k, ns],
                        start=(k == 0),
                        stop=(k == KC - 1),
                    )
                # RoPE: rot = ps*cos + swap16(ps)*sin_signed — all DVE, fp32;
                # gpsimd stays single-purpose (partition broadcast) so it
                # never swaps ucode libraries
                qk_raw = work.tile([128, 512], F32, tag="qk_raw", bufs=3)
                swp = work.tile([128, 512], F32, tag="swp")
                nc.vector.tensor_copy(qk_raw[:], ps[:])
                nc.vector.stream_shuffle(swp[:], qk_raw[:], SWAP16)
                nc.vector.tensor_mul(qk_raw[:], qk_raw[:], cos_sb[:, ns])
                nc.vector.tensor_mul(swp[:], swp[:], sin_sb[:, ns])
                nc.vector.tensor_add(qkT_sb[:, m, ns], qk_raw[:], swp[:])

            def do_v_tile(n, j):
                ps = ps_mm.tile([128, 256], F32, tag="mm")
                for k in range(KC):
                    nc.tensor.matmul(
                        ps[:],
                        lhsT=xT_sb[:, k, j * 128:(j + 1) * 128],
                        rhs=wv_sb[:, k, :],
                        start=(k == 0),
                        stop=(k == KC - 1),
                    )
                nc.vector.tensor_copy(
                    v_sb[:, j, :, 0:HD], ps[:].rearrange("p (h d) -> p h d", h=HPG)
                )

            def do_proj_tile(t, nh):
                ps = ps_mm.tile([128, 512], F32, tag="mm")
                for c in range(2):
                    nc.tensor.matmul(
                        ps[:],
                        lhsT=outT_sb[:, c, t * 128:(t + 1) * 128],
                        rhs=wout_sb[:, c, nh * 512:(nh + 1) * 512],
                        start=(c == 0),
                        stop=(c == 1),
                    )
                ot = outp.tile([128, 512], F32, tag="ot")
                if (t + nh) % 2 == 0:
                    nc.scalar.copy(out=ot[:], in_=ps[:])
                else:
                    nc.vector.tensor_copy(ot[:], ps[:])
                nc.sync.dma_start(
                    out=out_d[t * 128:(t + 1) * 128, nh * 512:(nh + 1) * 512],
                    in_=ot[:],
                )

            # ---- qkv chunk 0 head-pair 0 up front (attn group 0 hp=0 needs
            # m=0,2 + v tiles; m=1,3 for hp=1 go in as early fill) ----
            do_qk_tile(0, 0)
            do_qk_tile(0, 2)
            for j in range(4):
                do_v_tile(0, j)

            # fill units injected into attention groups, balanced so the
            # ACT-heavy late groups carry less fill
            fills = {
                0: [lambda: do_qk_tile(0, 1), lambda: do_qk_tile(0, 3)]
                 + [lambda m=m: do_qk_tile(1, m) for m in range(4)]
                 + [lambda j=j: do_v_tile(1, j) for j in range(4, 8)],
                1: [lambda m=m: do_qk_tile(2, m) for m in range(4)]
                 + [lambda j=j: do_v_tile(2, j) for j in range(8, 12)],
                2: [lambda m=m: do_qk_tile(3, m) for m in range(4)]
                 + [lambda j=j: do_v_tile(3, j) for j in range(12, 16)]
                 + [lambda t=t, nh=nh: do_proj_tile(t, nh)
                    for t in range(0, 4) for nh in range(2)],
                3: [lambda t=t, nh=nh: do_proj_tile(t, nh)
                    for t in range(4, 12) for nh in range(2)],
            }

            def make_pv(g, hp, j, jmax, ex, pv0, pv1, ncols, nstart):
                def issue():
                    for half, pv in ((0, pv0), (1, pv1)):
                        nc.tensor.matmul(
                            pv[:, nstart:512],
                            lhsT=v_sb[:, j, 2 * hp + half, :],
                            rhs=ex[:, half * 512:half * 512 + ncols],
                            start=(j == 0),
                            stop=(j == jmax),
                        )
                return issue

            def make_fin(g, hp, pv0, pv1):
                def issue():
                    for half, pv in ((0, pv0), (1, pv1)):
                        pb = 64 * half
                        den = work.tile([1, 512], F32, tag="den", bufs=1)
                        nc.vector.tensor_copy(den[:], pv[64:65, :])
                        rec = work.tile([1, 512], F32, tag="rec", bufs=1)
                        nc.vector.reciprocal_approx_fast(rec[:], den[:])
                        recb = work.tile([64, 512], F32, tag="recb", bufs=1)
                        nc.gpsimd.partition_broadcast(recb[:], rec[0:1, :], channels=64)
                        nc.vector.tensor_mul(
                            outT_sb[pb:pb + 64, hp, g * 512:(g + 1) * 512],
                            pv[0:64, :],
                            recb[:],
                        )
                return issue

            pend = []  # closures from the previous slot (PV pair, maybe fin)
            for g in range(4):
                fl = fills[g]
                slots = [(hp, j) for hp in range(2) for j in range(4 * g + 4)]
                fi = 0
                pv_state = {}
                for idx, (hp, j) in enumerate(slots):
                    qm, km = hp, 2 + hp
                    jmax = 4 * g + 3
                    if j == 0:
                        pv0_t = ps_pv.tile([65, 512], F32, tag="pv0", name="pv0")
                        pv1_t = ps_pv.tile([65, 512], F32, tag="pv1", name="pv1")
                        pv_state[hp] = (pv0_t, pv1_t)
                    pv0, pv1 = pv_state[hp]
                    d = j - 4 * g
                    nstart = 128 * d if d > 0 else 0
                    ncols = 512 - nstart
                    ex = expp.tile([128, 1024], BF16, tag="ex")
                    # two heads' score matmuls packed into one PE pass
                    # (row groups 0-1 / 2-3), one wide exp over both
                    sc = ps_sc.tile([128, 1024], F32, tag="sc")
                    for half in range(2):
                        pb = 64 * half
                        nc.tensor.matmul(
                            sc[:, half * 512:half * 512 + ncols],
                            lhsT=qkT_sb[pb:pb + 64, km, j * 128:(j + 1) * 128],
                            rhs=qkT_sb[pb:pb + 64, qm, g * 512 + nstart:(g + 1) * 512],
                            start=True,
                            stop=True,
                        )
                    if ncols == 512:
                        nc.scalar.activation(
                            ex[:], sc[:],
                            mybir.ActivationFunctionType.Exp, scale=0.125,
                        )
                    else:
                        exv = ex[:].rearrange("p (u c) -> p u c", u=2)[:, :, 0:ncols]
                        scv = sc[:].rearrange("p (u c) -> p u c", u=2)[:, :, 0:ncols]
                        nc.scalar.activation(
                            exv, scv, mybir.ActivationFunctionType.Exp, scale=0.125,
                        )
                    if d >= 0:
                        nc.vector.tensor_mul(ex[:, 0:128], ex[:, 0:128], tri_sb[:])
                        nc.vector.tensor_mul(ex[:, 512:640], ex[:, 512:640], tri_sb[:])
                    # fill the PE stream while ACT computes this block's exp
                    while fi < len(fl) and fi <= idx * len(fl) // len(slots):
                        fl[fi]()
                        fi += 1
                    # previous slot's PV (its exp has had a full slot to finish)
                    for c in pend:
                        c()
                    pend = [make_pv(g, hp, j, jmax, ex, pv0, pv1, ncols, nstart)]
                    if j == jmax:
                        pend.append(make_fin(g, hp, pv0, pv1))
                while fi < len(fl):
                    fl[fi]()
                    fi += 1
            for c in pend:
                c()
            for t in range(12, 16):
                for nh in range(2):
                    do_proj_tile(t, nh)

    nc.compile()
    return nc


def _qk_perm():
    """hd permutation for q/k columns: RoPE pair j -> (e,o) rows 16-interleaved
    so the swap stays within 32-partition quadrants (stream_shuffle-able)."""
    perm = np.empty(HD, dtype=np.int64)
    for p in range(HD):
        q32, i = divmod(p, 32)
        j = 16 * q32 + (i % 16)
        perm[p] = 2 * j + (1 if i >= 16 else 0)
    return perm


def _prepare_shards(x, w_qkv, w_out, freqs_cos, freqs_sin):
    perm = _qk_perm()
    cosT = np.ascontiguousarray(freqs_cos.T)  # [32, T]
    sinT = np.ascontiguousarray(freqs_sin.T)
    # row p of a 64-row head block: pair j = 16*(p//32 % 2) + p%16, sign -/+ for e/o
    cos128 = np.empty((128, T), dtype=np.float32)
    sin128s = np.empty((128, T), dtype=np.float32)
    for p in range(128):
        ph = p % 64
        q32, i = divmod(ph, 32)
        j = 16 * q32 + (i % 16)
        cos128[p] = cosT[j]
        sin128s[p] = sinT[j] * (-1.0 if i < 16 else 1.0)
    kk, qq = np.meshgrid(np.arange(128), np.arange(128), indexing="ij")
    tri = (kk <= qq).astype(ml_dtypes.bfloat16)

    w3 = w_qkv.reshape(D, 3, H, HD)
    in_maps = []
    for core in range(8):
        b, g = divmod(core, G)
        heads = np.arange(g * HPG, (g + 1) * HPG)
        wq = w3[:, 0, heads][:, :, perm].reshape(D, DG)
        wk = w3[:, 1, heads][:, :, perm].reshape(D, DG)
        wqk = np.ascontiguousarray(np.concatenate([wq, wk], axis=1))
        wv = np.ascontiguousarray(w3[:, 2, heads].reshape(D, DG))
        wo = np.ascontiguousarray(w_out.reshape(H, HD, D)[heads].reshape(DG, D))
        def sb_layout(a, kc=KC):
            # [128*kc, F] -> [128, kc, F] with partition-major contiguity
            return np.ascontiguousarray(
                a.reshape(kc, 128, -1).transpose(1, 0, 2)
            ).astype(ml_dtypes.bfloat16)
        def xT_layout(a):
            # [D, T] -> [128, NT, KC, 512]: T-chunks contiguous per partition
            t = a.reshape(KC, 128, NT, 512)
            return np.ascontiguousarray(
                t.transpose(1, 2, 0, 3)
            ).astype(ml_dtypes.bfloat16)
        in_maps.append({
            "xT": xT_layout(x[b].T),
            "wqk": sb_layout(wqk),
            "wv": sb_layout(wv),
            "wout": sb_layout(wo, kc=2),
            "cos128": cos128,
            "sin128s": sin128s,
            "tri": tri,
        })
    return in_maps


def _run(in_maps, **kw):
    if "nc" not in _CACHE:
        _CACHE["nc"] = _build()
    return run_bass_kernel_spmd(_CACHE["nc"], in_maps, core_ids=list(range(8)), **kw)


def kernel(x, w_qkv, w_out, freqs_cos, freqs_sin):
    x = np.asarray(x, dtype=np.float32)
    w_qkv = np.asarray(w_qkv, dtype=np.float32)
    w_out = np.asarray(w_out, dtype=np.float32)
    freqs_cos = np.asarray(freqs_cos, dtype=np.float32)
    freqs_sin = np.asarray(freqs_sin, dtype=np.float32)

    in_maps = _prepare_shards(x, w_qkv, w_out, freqs_cos, freqs_sin)
    res = _run(in_maps)
    out = np.zeros((B, T, D), dtype=np.float64)
    for core in range(8):
        out[core // G] += res.results[core]["out"].astype(np.float64)
    return out.astype(np.float32)


# revision 23
# speedup vs baseline: 1.0566x; 1.0523x over previous
"""Causal self-attention (B=2,T=2048,D=1024,H=16,HD=64) + RoPE on 8 TRN2 NeuronCores.

Sharding: core = b*4 + g  (b: batch, g: head-group of 4 heads).
Each core computes QKV projection for its 4 heads, causal attention, and a
partial out-projection (rank-256 contribution). Host sums the 4 partials per
batch (the "all-reduce after out_proj").

All matmul operands are bf16 (PSUM accumulation in fp32): bf16 streams at
1 col/cycle on the PE vs 2 for fp32, and enables fast weight loads.

The attention inner loop is software-pipelined at block granularity: PV(j-1)
is issued one iteration late so the PE never waits on the ACT exp, and QKV /
out-projection matmul "fill units" are injected between score and PV issues
to keep the PE stream dense (no HAM re-throttle) while ACT drains exps.
"""
import numpy as np
import ml_dtypes

import concourse.bass as bass
import concourse.mybir as mybir
from concourse import bacc
from concourse.tile import TileContext
from concourse.bass_utils import run_bass_kernel_spmd

B, T, D, H = 2, 2048, 1024, 16
HD = D // H            # 64
G = 4                  # head groups (tensor-parallel factor)
HPG = H // G           # 4 heads per group
DG = HPG * HD          # 256 head-dims per group
KC = D // 128          # 8 contraction chunks for D
NT = T // 512          # 4 T-chunks of 512
TT = T // 128          # 16 T-tiles of 128
F32 = mybir.dt.float32
BF16 = mybir.dt.bfloat16
SWAP16 = [(i + 16) % 32 for i in range(32)]  # e<->o halves within each 32-quadrant

_CACHE = {}


def _build():
    nc = bacc.Bacc("TRN2", target_bir_lowering=False, debug=False, num_devices=8)

    xT_d = nc.dram_tensor("xT", [128, NT, KC, 512], BF16, kind="ExternalInput").ap()
    wqk_d = nc.dram_tensor("wqk", [128, KC, 2 * DG], BF16, kind="ExternalInput").ap()
    wv_d = nc.dram_tensor("wv", [128, KC, DG], BF16, kind="ExternalInput").ap()
    wout_d = nc.dram_tensor("wout", [128, 2, D], BF16, kind="ExternalInput").ap()
    cos_d = nc.dram_tensor("cos128", [128, T], F32, kind="ExternalInput").ap()
    sin_d = nc.dram_tensor("sin128s", [128, T], F32, kind="ExternalInput").ap()
    tri_d = nc.dram_tensor("tri", [128, 128], BF16, kind="ExternalInput").ap()
    out_d = nc.dram_tensor("out", [T, D], F32, kind="ExternalOutput").ap()

    with TileContext(nc) as tc:
        with (
            tc.tile_pool(name="const", bufs=1) as cpool,
            tc.tile_pool(name="big", bufs=1) as big,
            tc.tile_pool(name="work", bufs=2) as work,
            tc.tile_pool(name="expp", bufs=4) as expp,
            tc.tile_pool(name="outp", bufs=2) as outp,
            tc.tile_pool(name="ps_mm", bufs=2, space="PSUM") as ps_mm,
            tc.tile_pool(name="ps_sc", bufs=2, space="PSUM") as ps_sc,
            tc.tile_pool(name="ps_pv", bufs=1, space="PSUM") as ps_pv,
        ):
            cos_sb = cpool.tile([128, T], F32)
            sin_sb = cpool.tile([128, T], F32)
            tri_sb = cpool.tile([128, 128], BF16)
            xT_sb = big.tile([128, KC, T], BF16)
            wqk_sb = big.tile([128, KC, 2 * DG], BF16)
            wv_sb = big.tile([128, KC, DG], BF16)
            wout_sb = big.tile([128, 2, D], BF16)
            # inputs arrive pre-arranged in SBUF layout; first q/k matmuls
            # need wqk + xT chunk 0 (contiguous in DRAM), then RoPE needs
            # cos/sin — everything else after
            nc.sync.dma_start(out=wqk_sb[:], in_=wqk_d)
            nc.sync.dma_start(out=xT_sb[:, :, 0:512], in_=xT_d[:, 0])
            nc.sync.dma_start(out=wv_sb[:], in_=wv_d)
            nc.sync.dma_start(out=cos_sb[:], in_=cos_d[:])
            nc.sync.dma_start(out=sin_sb[:], in_=sin_d[:])
            nc.sync.dma_start(out=tri_sb[:], in_=tri_d)
            for n in range(1, NT):
                nc.sync.dma_start(
                    out=xT_sb[:, :, n * 512:(n + 1) * 512],
                    in_=xT_d[:, n],
                )
            nc.sync.dma_start(out=wout_sb[:], in_=wout_d)

            # PE warm-up: dummy matmuls fill the DMA lead-in so HAM unthrottles
            # before the first real matmul
            warm_sb = cpool.tile([128, 256], BF16)
            nc.vector.memset(warm_sb[:], 0.0)
            for w in range(46):
                wp = ps_sc.tile([128, 256], F32, tag="sc")
                nc.tensor.matmul(
                    wp[:], lhsT=warm_sb[:, 0:128], rhs=warm_sb[:],
                    start=True, stop=True,
                )

            # qkT_sb m-index: 0,1 = q head-pairs (0,1),(2,3); 2,3 = k pairs
            qkT_sb = big.tile([128, 4, T], BF16)
            v_sb = big.tile([128, TT, HPG, HD + 1], BF16)
            nc.vector.memset(v_sb[:, :, :, HD], 1.0)
            outT_sb = big.tile([128, 2, T], BF16)

            def do_qk_tile(n, m):
                ns = slice(n * 512, (n + 1) * 512)
                ps = ps_mm.tile([128, 512], F32, tag="mm")
                for k in range(KC):
                    nc.tensor.matmul(
                        ps[:],
                        lhsT=wqk_sb[:, k, m * 128:(m + 1) * 128],
                        rhs=xT_sb[:, k, ns],
                        start=(k == 0),
                        stop=(k == KC - 1),
                    )
                # RoPE: rot = ps*cos + swap16(ps)*sin_signed — all DVE, fp32;
                # gpsimd stays single-purpose (partition broadcast) so it
                # never swaps ucode libraries
                qk_raw = work.tile([128, 512], F32, tag="qk_raw", bufs=3)
                swp = work.tile([128, 512], F32, tag="swp")
                nc.scalar.copy(out=qk_raw[:], in_=ps[:])
                nc.vector.stream_shuffle(swp[:], qk_raw[:], SWAP16)
                nc.vector.tensor_mul(qk_raw[:], qk_raw[:], cos_sb[:, ns])
                nc.vector.tensor_mul(swp[:], swp[:], sin_sb[:, ns])
                nc.vector.tensor_add(qkT_sb[:, m, ns], qk_raw[:], swp[:])

            def do_v_tile(n, j):
                ps = ps_mm.tile([128, 256], F32, tag="mm")
                for k in range(KC):
                    nc.tensor.matmul(
                        ps[:],
                        lhsT=xT_sb[:, k, j * 128:(j + 1) * 128],
                        rhs=wv_sb[:, k, :],
                        start=(k == 0),
                        stop=(k == KC - 1),
                    )
                nc.vector.tensor_copy(
                    v_sb[:, j, :, 0:HD], ps[:].rearrange("p (h d) -> p h d", h=HPG)
                )

            def do_proj_tile(t, nh):
                ps = ps_mm.tile([128, 512], F32, tag="mm")
                for c in range(2):
                    nc.tensor.matmul(
                        ps[:],
                        lhsT=outT_sb[:, c, t * 128:(t + 1) * 128],
                        rhs=wout_sb[:, c, nh * 512:(nh + 1) * 512],
                        start=(c == 0),
                        stop=(c == 1),
                    )
                ot = outp.tile([128, 512], F32, tag="ot")
                if (t + nh) % 2 == 0:
                    nc.scalar.copy(out=ot[:], in_=ps[:])
                else:
                    nc.vector.tensor_copy(ot[:], ps[:])
                nc.sync.dma_start(
                    out=out_d[t * 128:(t + 1) * 128, nh * 512:(nh + 1) * 512],
                    in_=ot[:],
                )

            # ---- qkv chunk 0 head-pair 0 up front (attn group 0 hp=0 needs
            # m=0,2 + v tiles; m=1,3 for hp=1 go in as early fill) ----
            do_qk_tile(0, 0)
            do_qk_tile(0, 2)
            for j in range(4):
                do_v_tile(0, j)

            # fill units injected into attention groups, balanced so the
            # ACT-heavy late groups carry less fill
            fills = {
                0: [lambda: do_qk_tile(0, 1), lambda: do_qk_tile(0, 3)]
                 + [lambda m=m: do_qk_tile(1, m) for m in range(4)]
                 + [lambda j=j: do_v_tile(1, j) for j in range(4, 8)],
                1: [lambda m=m: do_qk_tile(2, m) for m in range(4)]
                 + [lambda j=j: do_v_tile(2, j) for j in range(8, 12)],
                2: [lambda m=m: do_qk_tile(3, m) for m in range(4)]
                 + [lambda j=j: do_v_tile(3, j) for j in range(12, 16)]
                 + [lambda t=t, nh=nh: do_proj_tile(t, nh)
                    for t in range(0, 4) for nh in range(2)],
                3: [lambda t=t, nh=nh: do_proj_tile(t, nh)
                    for t in range(4, 12) for nh in range(2)],
            }

            def make_pv(g, hp, j, jmax, ex, pv0, pv1, ncols, nstart):
                def issue():
                    for half, pv in ((0, pv0), (1, pv1)):
                        nc.tensor.matmul(
                            pv[:, nstart:512],
                            lhsT=v_sb[:, j, 2 * hp + half, :],
                            rhs=ex[:, half * 512:half * 512 + ncols],
                            start=(j == 0),
                            stop=(j == jmax),
                        )
                return issue

            def make_fin(g, hp, pv0, pv1):
                def issue():
                    for half, pv in ((0, pv0), (1, pv1)):
                        pb = 64 * half
                        den = work.tile([1, 512], F32, tag="den", bufs=1)
                        nc.vector.tensor_copy(den[:], pv[64:65, :])
                        rec = work.tile([1, 512], F32, tag="rec", bufs=1)
                        nc.vector.reciprocal_approx_fast(rec[:], den[:])
                        recb = work.tile([64, 512], F32, tag="recb", bufs=1)
                        nc.gpsimd.partition_broadcast(recb[:], rec[0:1, :], channels=64)
                        nc.vector.tensor_mul(
                            outT_sb[pb:pb + 64, hp, g * 512:(g + 1) * 512],
                            pv[0:64, :],
                            recb[:],
                        )
                return issue

            pend = []  # closures from the previous slot (PV pair, maybe fin)
            for g in range(4):
                fl = fills[g]
                slots = [(hp, j) for hp in range(2) for j in range(4 * g + 4)]
                fi = 0
                pv_state = {}
                for idx, (hp, j) in enumerate(slots):
                    qm, km = hp, 2 + hp
                    jmax = 4 * g + 3
                    if j == 0:
                        pv0_t = ps_pv.tile([65, 512], F32, tag="pv0", name="pv0")
                        pv1_t = ps_pv.tile([65, 512], F32, tag="pv1", name="pv1")
                        pv_state[hp] = (pv0_t, pv1_t)
                    pv0, pv1 = pv_state[hp]
                    d = j - 4 * g
                    nstart = 128 * d if d > 0 else 0
                    ncols = 512 - nstart
                    ex = expp.tile([128, 1024], BF16, tag="ex")
                    # two heads' score matmuls packed into one PE pass
                    # (row groups 0-1 / 2-3), one wide exp over both
                    sc = ps_sc.tile([128, 1024], F32, tag="sc")
                    for half in range(2):
                        pb = 64 * half
                        nc.tensor.matmul(
                            sc[:, half * 512:half * 512 + ncols],
                            lhsT=qkT_sb[pb:pb + 64, km, j * 128:(j + 1) * 128],
                            rhs=qkT_sb[pb:pb + 64, qm, g * 512 + nstart:(g + 1) * 512],
                            start=True,
                            stop=True,
                        )
                    if ncols == 512:
                        nc.scalar.activation(
                            ex[:], sc[:],
                            mybir.ActivationFunctionType.Exp, scale=0.125,
                        )
                    else:
                        exv = ex[:].rearrange("p (u c) -> p u c", u=2)[:, :, 0:ncols]
                        scv = sc[:].rearrange("p (u c) -> p u c", u=2)[:, :, 0:ncols]
                        nc.scalar.activation(
                            exv, scv, mybir.ActivationFunctionType.Exp, scale=0.125,
                        )
                    if d >= 0:
                        nc.vector.tensor_mul(ex[:, 0:128], ex[:, 0:128], tri_sb[:])
                        nc.vector.tensor_mul(ex[:, 512:640], ex[:, 512:640], tri_sb[:])
                    # fill the PE stream while ACT computes this block's exp
                    while fi < len(fl) and fi <= idx * len(fl) // len(slots):
                        fl[fi]()
                        fi += 1
                    # previous slot's PV (its exp has had a full slot to finish)
                    for c in pend:
                        c()
                    pend = [make_pv(g, hp, j, jmax, ex, pv0, pv1, ncols, nstart)]
                    if j == jmax:
                        pend.append(make_fin(g, hp, pv0, pv1))
                while fi < len(fl):
                    fl[fi]()
                    fi += 1
            for c in pend:
                c()
            for t in range(12, 16):
                for nh in range(2):
                    do_proj_tile(t, nh)

    nc.compile()
    return nc


def _qk_perm():
    """hd permutation for q/k columns: RoPE pair j -> (e,o) rows 16-interleaved
    so the swap stays within 32-partition quadrants (stream_shuffle-able)."""
    perm = np.empty(HD, dtype=np.int64)
    for p in range(HD):
        q32, i = divmod(p, 32)
        j = 16 * q32 + (i % 16)
        perm[p] = 2 * j + (1 if i >= 16 else 0)
    return perm


def _prepare_shards(x, w_qkv, w_out, freqs_cos, freqs_sin):
    perm = _qk_perm()
    cosT = np.ascontiguousarray(freqs_cos.T)  # [32, T]
    sinT = np.ascontiguousarray(freqs_sin.T)
    # row p of a 64-row head block: pair j = 16*(p//32 % 2) + p%16, sign -/+ for e/o
    cos128 = np.empty((128, T), dtype=np.float32)
    sin128s = np.empty((128, T), dtype=np.float32)
    for p in range(128):
        ph = p % 64
        q32, i = divmod(ph, 32)
        j = 16 * q32 + (i % 16)
        cos128[p] = cosT[j]
        sin128s[p] = sinT[j] * (-1.0 if i < 16 else 1.0)
    kk, qq = np.meshgrid(np.arange(128), np.arange(128), indexing="ij")
    tri = (kk <= qq).astype(ml_dtypes.bfloat16)

    w3 = w_qkv.reshape(D, 3, H, HD)
    in_maps = []
    for core in range(8):
        b, g = divmod(core, G)
        heads = np.arange(g * HPG, (g + 1) * HPG)
        wq = w3[:, 0, heads][:, :, perm].reshape(D, DG)
        wk = w3[:, 1, heads][:, :, perm].reshape(D, DG)
        wqk = np.ascontiguousarray(np.concatenate([wq, wk], axis=1))
        wv = np.ascontiguousarray(w3[:, 2, heads].reshape(D, DG))
        wo = np.ascontiguousarray(w_out.reshape(H, HD, D)[heads].reshape(DG, D))
        def sb_layout(a, kc=KC):
            # [128*kc, F] -> [128, kc, F] with partition-major contiguity
            return np.ascontiguousarray(
                a.reshape(kc, 128, -1).transpose(1, 0, 2)
            ).astype(ml_dtypes.bfloat16)
        def xT_layout(a):
            # [D, T] -> [128, NT, KC, 512]: T-chunks contiguous per partition
            t = a.reshape(KC, 128, NT, 512)
            return np.ascontiguousarray(
                t.transpose(1, 2, 0, 3)
            ).astype(ml_dtypes.bfloat16)
        in_maps.append({
            "xT": xT_layout(x[b].T),
            "wqk": sb_layout(wqk),
            "wv": sb_layout(wv),
            "wout": sb_layout(wo, kc=2),
            "cos128": cos128,
            "sin128s": sin128s,
            "tri": tri,
        })
    return in_maps


def _run(in_maps, **kw):
    if "nc" not in _CACHE:
        _CACHE["nc"] = _build()
    return run_bass_kernel_spmd(_CACHE["nc"], in_maps, core_ids=list(range(8)), **kw)


def kernel(x, w_qkv, w_out, freqs_cos, freqs_sin):
    x = np.asarray(x, dtype=np.float32)
    w_qkv = np.asarray(w_qkv, dtype=np.float32)
    w_out = np.asarray(w_out, dtype=np.float32)
    freqs_cos = np.asarray(freqs_cos, dtype=np.float32)
    freqs_sin = np.asarray(freqs_sin, dtype=np.float32)

    in_maps = _prepare_shards(x, w_qkv, w_out, freqs_cos, freqs_sin)
    res = _run(in_maps)
    out = np.zeros((B, T, D), dtype=np.float64)
    for core in range(8):
        out[core // G] += res.results[core]["out"].astype(np.float64)
    return out.astype(np.float32)
